# revision 5
# baseline (speedup 1.0000x reference)
"""CondConv2d (soft mixture-of-experts 3x3 conv) — Trainium2 Bass kernel.

Reference computation:
    pooled  = x.mean((2,3))                      # [n, c]
    routing = sigmoid(pooled @ routing_ws)       # [n, E]
    ws      = einsum('ne,eoihw->noihw', routing, weight)
    y[n]    = conv2d(x[n], ws[n], pad=1)         # [n, oc, h, w]

Sharding: data-parallel over batch. 8 cores x 4 samples each; routing_ws and
the expert weight bank are replicated. Everything is computed on-device.

Per-core pipeline (per sample n):
  load:    DMA x[n] fp32 -> SBUF; ACT casts to bf16 into a W-padded
           [128, 56, 58] tile, with accum_out producing the spatial sum
           (pooled) in the same pass.  Casts run two samples ahead.
  routing: PE logits[1,8] = pooled.T @ routing_ws; ACT sigmoid(/3136);
           PE ones-matmul broadcasts the 8 gates to 128 partitions; ACT copy.
           Issued one sample ahead of the conv stream.
  mixing:  wmix = sum_e g[e]*W_e per 128-channel input group. Two ACT gated
           muls seed two accumulators; DVE alternates tensor_scalar (4x) and
           tensor_tensor add (2x); final add combines.  Issued one sample
           ahead so it overlaps the previous sample's convs.
  conv:    9 taps x 2 input groups = 18 matmuls accumulate into one PSUM bank
           per (output group, 8-row tile); H edges via clipped row ranges,
           W edges via the 58-wide zero-padded layout. ACT copies PSUM->SBUF,
           DMA out.  Sample 0 runs icg0 taps for 6 banks before icg1 taps so
           the PE starts as soon as the first mixing chain lands.
"""

import numpy as np

N, C, H, W = 32, 256, 56, 56
E, OC, IC, FH, FW = 8, 256, 256, 3, 3
NCORES = 8
NS = N // NCORES          # samples per core
P = 128                   # partitions
ICG = IC // P             # input-channel groups (2)
OCG = OC // P             # output-channel groups (2)
WP = W + 2                # padded width (58)
RROWS = 8                 # output rows per PSUM tile
NRT = H // RROWS          # row tiles (7)
NTAP = FH * FW
# dy=1 taps first so the start=True matmul covers the full 8-row PSUM tile
TAP_ORDER = [(1, 0), (1, 1), (1, 2), (0, 0), (0, 1), (0, 2), (2, 0), (2, 1), (2, 2)]

_cache = {}


def _build():
    from concourse import bacc, mybir
    import concourse.tile as tile

    f32 = mybir.dt.float32
    bf16 = mybir.dt.bfloat16
    ADD = mybir.AluOpType.add
    Copy = mybir.ActivationFunctionType.Copy
    Sigmoid = mybir.ActivationFunctionType.Sigmoid

    nc = bacc.Bacc("TRN2", target_bir_lowering=False, debug=False, num_devices=NCORES)
    x_d = nc.dram_tensor("x", [NS, C, H, W], f32, kind="ExternalInput").ap()
    wt_d = nc.dram_tensor("wt", [E, ICG, P, NTAP * OC], bf16, kind="ExternalInput").ap()
    rws_d = nc.dram_tensor("rws", [ICG, P, E], f32, kind="ExternalInput").ap()
    y_d = nc.dram_tensor("y", [NS, OC, H, W], f32, kind="ExternalOutput").ap()

    with tile.TileContext(nc) as tc:
        with (
            tc.tile_pool(name="consts", bufs=1) as consts,
            tc.tile_pool(name="wbank", bufs=1) as wbank_pool,
            tc.tile_pool(name="xf", bufs=2) as xf_pool,
            tc.tile_pool(name="xb", bufs=6) as xb_pool,
            tc.tile_pool(name="small", bufs=4) as small,
            tc.tile_pool(name="wmix", bufs=4) as wmix_pool,
            tc.tile_pool(name="scratch", bufs=2) as scratch,
            tc.tile_pool(name="outp", bufs=4) as out_pool,
            tc.tile_pool(name="cpsum", bufs=6, space="PSUM") as cpsum,
            tc.tile_pool(name="rpsum", bufs=1, space="PSUM") as rpsum,
        ):
            xf_t = [None] * NS
            xb_t = [[None] * ICG for _ in range(NS)]
            pooled = [[None] * ICG for _ in range(NS)]
            g_t = [None] * NS
            wmix = [[None] * ICG for _ in range(NS)]

            def load_x(n):
                # x loads ride the ACT HWDGE ring so they run in parallel
                # with the weight-bank DMAs on the sync ring
                xf_t[n] = []
                for icg in range(ICG):
                    xf = xf_pool.tile([P, H * W], f32, name="xf")
                    nc.scalar.dma_start(xf[:], x_d[n, icg * P:(icg + 1) * P])
                    xf_t[n].append(xf)

            def cast_x(n):
                for icg in range(ICG):
                    xbt = xb_pool.tile([P, H, WP], bf16, name="xbt")
                    nc.gpsimd.memset(xbt[:, :, 0:1], 0.0)
                    nc.gpsimd.memset(xbt[:, :, W + 1:W + 2], 0.0)
                    pt = small.tile([P, 1], f32, name="pt")
                    nc.scalar.activation(
                        xbt[:, :, 1:W + 1],
                        xf_t[n][icg].rearrange("p (h w) -> p h w", h=H),
                        Copy, accum_out=pt[:],
                    )
                    xb_t[n][icg] = xbt
                    pooled[n][icg] = pt

            def routing(n):
                logits = rpsum.tile([1, E], f32, name="logits")
                for icg in range(ICG):
                    nc.tensor.matmul(
                        logits[:], pooled[n][icg][:], rws_sb[icg][:],
                        start=(icg == 0), stop=(icg == ICG - 1),
                    )
                lg = small.tile([1, E], f32, name="lg")
                nc.scalar.activation(lg[:], logits[:], Sigmoid, scale=1.0 / (H * W))
                gps = rpsum.tile([P, E], f32, name="gps")
                nc.tensor.matmul(gps[:], ones_sb[:], lg[:], start=True, stop=True)
                g = small.tile([P, E], f32, name="g")
                nc.scalar.copy(g[:], gps[:])
                g_t[n] = g

            def mixing(n):
                g = g_t[n]
                for icg in range(ICG):
                    wm = wmix_pool.tile([P, NTAP * OC], bf16, name="wm")
                    accb = scratch.tile([P, NTAP * OC], bf16, name="accb")
                    nc.scalar.mul(wm[:], wbank[0][icg][:], g[:, 0:1])
                    nc.scalar.mul(accb[:], wbank[1][icg][:], g[:, 1:2])
                    for e in range(2, E):
                        p = scratch.tile([P, NTAP * OC], bf16, name="p", bufs=2)
                        nc.vector.tensor_scalar_mul(p[:], wbank[e][icg][:], g[:, e:e + 1])
                        tgt = wm if e % 2 == 0 else accb
                        nc.vector.tensor_add(tgt[:], tgt[:], p[:])
                    nc.vector.tensor_add(wm[:], wm[:], accb[:])
                    wmix[n][icg] = wm

            def conv_mms(n, ocg, rt, ps, icg, is_first, is_last):
                r0 = rt * RROWS
                cnt = 0
                for dy, dx in TAP_ORDER:
                    lo = max(r0, 1 - dy)
                    hi = min(r0 + RROWS - 1, H - dy)
                    tap = dy * FW + dx
                    nc.tensor.matmul(
                        ps[:, lo - r0:hi - r0 + 1, :],
                        wmix[n][icg][:, tap * OC + ocg * P:tap * OC + ocg * P + P],
                        xb_t[n][icg][:, lo + dy - 1:hi + dy, dx:dx + W],
                        start=(is_first and cnt == 0),
                        stop=(is_last and cnt == NTAP - 1),
                    )
                    cnt += 1

            def bank_out(n, ocg, rt, ps):
                r0 = rt * RROWS
                ot = out_pool.tile([P, RROWS, W], f32, name="ot")
                nc.scalar.copy(ot[:], ps[:])
                nc.sync.dma_start(
                    y_d[n, ocg * P:(ocg + 1) * P, r0:r0 + RROWS], ot[:]
                )

            BANKS = [(ocg, rt) for ocg in range(OCG) for rt in range(NRT)]

            def conv_banks(n, banks):
                for ocg, rt in banks:
                    ps = cpsum.tile([P, RROWS, W], f32, name="ps")
                    conv_mms(n, ocg, rt, ps, 0, True, False)
                    conv_mms(n, ocg, rt, ps, 1, False, True)
                    bank_out(n, ocg, rt, ps)

            def conv_banks_icg_split(n, banks):
                # run icg0 taps for all these banks before any icg1 taps, so
                # the PE starts as soon as the first mixing chain lands
                tiles = []
                for ocg, rt in banks:
                    ps = cpsum.tile([P, RROWS, W], f32, name="ps")
                    conv_mms(n, ocg, rt, ps, 0, True, False)
                    tiles.append(ps)
                for (ocg, rt), ps in zip(banks, tiles):
                    conv_mms(n, ocg, rt, ps, 1, False, True)
                    bank_out(n, ocg, rt, ps)

            # --- prologue ---
            load_x(0)
            load_x(1)
            rws_sb = []
            for icg in range(ICG):
                rws_t = consts.tile([P, E], f32, name=f"rws{icg}")
                nc.sync.dma_start(rws_t[:], rws_d[icg])
                rws_sb.append(rws_t)
            ones_sb = consts.tile([1, P], f32, name="ones")
            nc.vector.memset(ones_sb[:], 1.0)
            wbank = [[None] * ICG for _ in range(E)]
            for icg in range(ICG):
                for e in range(E):
                    wb = wbank_pool.tile([P, NTAP * OC], bf16, name=f"wb{e}_{icg}")
                    nc.sync.dma_start(wb[:], wt_d[e, icg])
                    wbank[e][icg] = wb
            cast_x(0)
            cast_x(1)
            routing(0)
            mixing(0)

            # --- main loop ---
            # First banks of convs(n) are issued before next-sample prep so
            # ACT's PSUM copies keep pace with the PE; sample 0 additionally
            # splits icg passes so convs start after one mixing chain.
            for n in range(NS):
                head = BANKS[:5]
                if n == 0:
                    conv_banks_icg_split(n, head)
                else:
                    conv_banks(n, head)
                if n + 1 < NS:
                    routing(n + 1)
                    mixing(n + 1)
                if n + 2 < NS:
                    load_x(n + 2)
                    cast_x(n + 2)
                conv_banks(n, BANKS[5:])
    nc.finalize()
    return nc


def _get_nc():
    if "nc" not in _cache:
        _cache["nc"] = _build()
    return _cache["nc"]


def _in_maps(x, routing_ws, weight):
    import ml_dtypes

    x = np.ascontiguousarray(np.asarray(x, dtype=np.float32))
    routing_ws = np.asarray(routing_ws, dtype=np.float32)
    weight = np.asarray(weight, dtype=np.float32)
    # [E, oc, ic, fh, fw] -> [E, ic, fh, fw, oc] -> [E, ICG, 128, 9*256], bf16
    wt = np.ascontiguousarray(weight.transpose(0, 2, 3, 4, 1)).reshape(
        E, ICG, P, NTAP * OC
    ).astype(ml_dtypes.bfloat16)
    rws = np.ascontiguousarray(routing_ws.reshape(ICG, P, E))
    return [
        {"x": np.ascontiguousarray(x[i * NS:(i + 1) * NS]), "wt": wt, "rws": rws}
        for i in range(NCORES)
    ]


def _run(in_maps, **kwargs):
    from concourse import bass_utils

    return bass_utils.run_bass_kernel_spmd(
        _get_nc(), in_maps, core_ids=list(range(NCORES)), **kwargs
    )


def kernel(x, routing_ws, weight):
    res = _run(_in_maps(x, routing_ws, weight))
    return np.concatenate([r["y"] for r in res.results], axis=0)


# revision 9
# speedup vs baseline: 1.0058x; 1.0058x over previous
"""CondConv2d (soft mixture-of-experts 3x3 conv) — Trainium2 Bass kernel.

Reference computation:
    pooled  = x.mean((2,3))                      # [n, c]
    routing = sigmoid(pooled @ routing_ws)       # [n, E]
    ws      = einsum('ne,eoihw->noihw', routing, weight)
    y[n]    = conv2d(x[n], ws[n], pad=1)         # [n, oc, h, w]

Sharding: data-parallel over batch. 8 cores x 4 samples each; routing_ws and
the expert weight bank are replicated. Everything is computed on-device.

Per-core pipeline (per sample n):
  load:    DMA x[n] fp32 -> SBUF; ACT casts to bf16 into a W-padded
           [128, 56, 58] tile, with accum_out producing the spatial sum
           (pooled) in the same pass.  Casts run two samples ahead.
  routing: PE logits[1,8] = pooled.T @ routing_ws; ACT sigmoid(/3136);
           PE ones-matmul broadcasts the 8 gates to 128 partitions; ACT copy.
           Issued one sample ahead of the conv stream.
  mixing:  wmix = sum_e g[e]*W_e per 128-channel input group. Two ACT gated
           muls seed two accumulators; DVE alternates tensor_scalar (4x) and
           tensor_tensor add (2x); final add combines.  Issued one sample
           ahead so it overlaps the previous sample's convs.
  conv:    9 taps x 2 input groups = 18 matmuls accumulate into one PSUM bank
           per (output group, 8-row tile); H edges via clipped row ranges,
           W edges via the 58-wide zero-padded layout. ACT copies PSUM->SBUF,
           DMA out.  Sample 0 runs icg0 taps for 6 banks before icg1 taps so
           the PE starts as soon as the first mixing chain lands.
"""

import numpy as np

N, C, H, W = 32, 256, 56, 56
E, OC, IC, FH, FW = 8, 256, 256, 3, 3
NCORES = 8
NS = N // NCORES          # samples per core
P = 128                   # partitions
ICG = IC // P             # input-channel groups (2)
OCG = OC // P             # output-channel groups (2)
WP = W + 2                # padded width (58)
RROWS = 8                 # output rows per PSUM tile
NRT = H // RROWS          # row tiles (7)
NTAP = FH * FW
# dy=1 taps first so the start=True matmul covers the full 8-row PSUM tile
TAP_ORDER = [(1, 0), (1, 1), (1, 2), (0, 0), (0, 1), (0, 2), (2, 0), (2, 1), (2, 2)]

_cache = {}


def _build():
    from concourse import bacc, mybir
    import concourse.tile as tile

    f32 = mybir.dt.float32
    bf16 = mybir.dt.bfloat16
    ADD = mybir.AluOpType.add
    Copy = mybir.ActivationFunctionType.Copy
    Sigmoid = mybir.ActivationFunctionType.Sigmoid

    nc = bacc.Bacc("TRN2", target_bir_lowering=False, debug=False, num_devices=NCORES)
    x_d = nc.dram_tensor("x", [NS, C, H, W], f32, kind="ExternalInput").ap()
    wt_d = nc.dram_tensor("wt", [E, ICG, P, NTAP * OC], bf16, kind="ExternalInput").ap()
    rws_d = nc.dram_tensor("rws", [ICG, P, E], f32, kind="ExternalInput").ap()
    y_d = nc.dram_tensor("y", [NS, OC, H, W], f32, kind="ExternalOutput").ap()

    with tile.TileContext(nc) as tc:
        with (
            tc.tile_pool(name="consts", bufs=1) as consts,
            tc.tile_pool(name="wbank", bufs=1) as wbank_pool,
            tc.tile_pool(name="xf", bufs=3) as xf_pool,
            tc.tile_pool(name="xb", bufs=6) as xb_pool,
            tc.tile_pool(name="small", bufs=4) as small,
            tc.tile_pool(name="wmix", bufs=4) as wmix_pool,
            tc.tile_pool(name="scratch", bufs=2) as scratch,
            tc.tile_pool(name="outp", bufs=4) as out_pool,
            tc.tile_pool(name="cpsum", bufs=6, space="PSUM") as cpsum,
            tc.tile_pool(name="rpsum", bufs=1, space="PSUM") as rpsum,
        ):
            xf_t = [None] * NS
            xb_t = [[None] * ICG for _ in range(NS)]
            pooled = [[None] * ICG for _ in range(NS)]
            g_t = [None] * NS
            wmix = [[None] * ICG for _ in range(NS)]

            def load_x(n):
                # x loads ride the ACT HWDGE ring so they run in parallel
                # with the weight-bank DMAs on the sync ring
                xf_t[n] = []
                for icg in range(ICG):
                    xf = xf_pool.tile([P, H * W], f32, name="xf")
                    nc.scalar.dma_start(xf[:], x_d[n, icg * P:(icg + 1) * P])
                    xf_t[n].append(xf)

            def cast_x(n):
                for icg in range(ICG):
                    xbt = xb_pool.tile([P, H, WP], bf16, name="xbt")
                    nc.gpsimd.memset(xbt[:, :, 0:1], 0.0)
                    nc.gpsimd.memset(xbt[:, :, W + 1:W + 2], 0.0)
                    pt = small.tile([P, 1], f32, name="pt")
                    nc.scalar.activation(
                        xbt[:, :, 1:W + 1],
                        xf_t[n][icg].rearrange("p (h w) -> p h w", h=H),
                        Copy, accum_out=pt[:],
                    )
                    xb_t[n][icg] = xbt
                    pooled[n][icg] = pt

            def routing(n):
                logits = rpsum.tile([1, E], f32, name="logits")
                for icg in range(ICG):
                    nc.tensor.matmul(
                        logits[:], pooled[n][icg][:], rws_sb[icg][:],
                        start=(icg == 0), stop=(icg == ICG - 1),
                    )
                lg = small.tile([1, E], f32, name="lg")
                nc.scalar.activation(lg[:], logits[:], Sigmoid, scale=1.0 / (H * W))
                gps = rpsum.tile([P, E], f32, name="gps")
                nc.tensor.matmul(gps[:], ones_sb[:], lg[:], start=True, stop=True)
                g = small.tile([P, E], f32, name="g")
                nc.scalar.copy(g[:], gps[:])
                g_t[n] = g

            def mixing(n):
                g = g_t[n]
                for icg in range(ICG):
                    wm = wmix_pool.tile([P, NTAP * OC], bf16, name="wm")
                    accb = scratch.tile([P, NTAP * OC], bf16, name="accb")
                    nc.scalar.mul(wm[:], wbank[0][icg][:], g[:, 0:1])
                    nc.scalar.mul(accb[:], wbank[1][icg][:], g[:, 1:2])
                    for e in range(2, E):
                        p = scratch.tile([P, NTAP * OC], bf16, name="p", bufs=2)
                        nc.vector.tensor_scalar_mul(p[:], wbank[e][icg][:], g[:, e:e + 1])
                        tgt = wm if e % 2 == 0 else accb
                        nc.vector.tensor_add(tgt[:], tgt[:], p[:])
                    nc.vector.tensor_add(wm[:], wm[:], accb[:])
                    wmix[n][icg] = wm

            def conv_mms(n, ocg, rt, ps, icg, is_first, is_last):
                r0 = rt * RROWS
                cnt = 0
                for dy, dx in TAP_ORDER:
                    lo = max(r0, 1 - dy)
                    hi = min(r0 + RROWS - 1, H - dy)
                    tap = dy * FW + dx
                    nc.tensor.matmul(
                        ps[:, lo - r0:hi - r0 + 1, :],
                        wmix[n][icg][:, tap * OC + ocg * P:tap * OC + ocg * P + P],
                        xb_t[n][icg][:, lo + dy - 1:hi + dy, dx:dx + W],
                        start=(is_first and cnt == 0),
                        stop=(is_last and cnt == NTAP - 1),
                    )
                    cnt += 1

            def bank_out(n, ocg, rt, ps):
                r0 = rt * RROWS
                ot = out_pool.tile([P, RROWS, W], f32, name="ot")
                nc.scalar.copy(ot[:], ps[:])
                nc.sync.dma_start(
                    y_d[n, ocg * P:(ocg + 1) * P, r0:r0 + RROWS], ot[:]
                )

            BANKS = [(ocg, rt) for ocg in range(OCG) for rt in range(NRT)]

            def conv_banks(n, banks):
                for ocg, rt in banks:
                    ps = cpsum.tile([P, RROWS, W], f32, name="ps")
                    conv_mms(n, ocg, rt, ps, 0, True, False)
                    conv_mms(n, ocg, rt, ps, 1, False, True)
                    bank_out(n, ocg, rt, ps)

            def conv_banks_icg_split(n, banks):
                # run icg0 taps for all these banks before any icg1 taps, so
                # the PE starts as soon as the first mixing chain lands
                tiles = []
                for ocg, rt in banks:
                    ps = cpsum.tile([P, RROWS, W], f32, name="ps")
                    conv_mms(n, ocg, rt, ps, 0, True, False)
                    tiles.append(ps)
                for (ocg, rt), ps in zip(banks, tiles):
                    conv_mms(n, ocg, rt, ps, 1, False, True)
                    bank_out(n, ocg, rt, ps)

            # --- prologue ---
            load_x(0)
            load_x(1)
            rws_sb = []
            for icg in range(ICG):
                rws_t = consts.tile([P, E], f32, name=f"rws{icg}")
                nc.sync.dma_start(rws_t[:], rws_d[icg])
                rws_sb.append(rws_t)
            ones_sb = consts.tile([1, P], f32, name="ones")
            nc.vector.memset(ones_sb[:], 1.0)
            wbank = [[None] * ICG for _ in range(E)]
            for icg in range(ICG):
                for e in range(E):
                    wb = wbank_pool.tile([P, NTAP * OC], bf16, name=f"wb{e}_{icg}")
                    nc.sync.dma_start(wb[:], wt_d[e, icg])
                    wbank[e][icg] = wb
            # routing(0)+mixing(0) must precede cast_x(1) in the ACT stream,
            # else the first sample's gates wait on sample 1's cast
            cast_x(0)
            routing(0)
            mixing(0)
            cast_x(1)

            # --- main loop ---
            # First banks of convs(n) are issued before next-sample prep so
            # ACT's PSUM copies keep pace with the PE; sample 0 additionally
            # splits icg passes so convs start after one mixing chain.
            for n in range(NS):
                if n == 0:
                    conv_banks_icg_split(n, BANKS[:6])
                    head_rest = BANKS[6:]
                else:
                    conv_banks(n, BANKS[:5])
                    head_rest = BANKS[5:]
                if n + 1 < NS:
                    routing(n + 1)
                    mixing(n + 1)
                if n + 2 < NS:
                    load_x(n + 2)
                    cast_x(n + 2)
                conv_banks(n, head_rest)
    nc.finalize()
    return nc


def _get_nc():
    if "nc" not in _cache:
        _cache["nc"] = _build()
    return _cache["nc"]


def _in_maps(x, routing_ws, weight):
    import ml_dtypes

    x = np.ascontiguousarray(np.asarray(x, dtype=np.float32))
    routing_ws = np.asarray(routing_ws, dtype=np.float32)
    weight = np.asarray(weight, dtype=np.float32)
    # [E, oc, ic, fh, fw] -> [E, ic, fh, fw, oc] -> [E, ICG, 128, 9*256], bf16
    wt = np.ascontiguousarray(weight.transpose(0, 2, 3, 4, 1)).reshape(
        E, ICG, P, NTAP * OC
    ).astype(ml_dtypes.bfloat16)
    rws = np.ascontiguousarray(routing_ws.reshape(ICG, P, E))
    return [
        {"x": np.ascontiguousarray(x[i * NS:(i + 1) * NS]), "wt": wt, "rws": rws}
        for i in range(NCORES)
    ]


def _run(in_maps, **kwargs):
    from concourse import bass_utils

    return bass_utils.run_bass_kernel_spmd(
        _get_nc(), in_maps, core_ids=list(range(NCORES)), **kwargs
    )


def kernel(x, routing_ws, weight):
    res = _run(_in_maps(x, routing_ws, weight))
    return np.concatenate([r["y"] for r in res.results], axis=0)


# revision 13
# speedup vs baseline: 1.0398x; 1.0338x over previous
"""CondConv2d (soft mixture-of-experts 3x3 conv) — Trainium2 Bass kernel.

Reference computation:
    pooled  = x.mean((2,3))                      # [n, c]
    routing = sigmoid(pooled @ routing_ws)       # [n, E]
    ws      = einsum('ne,eoihw->noihw', routing, weight)
    y[n]    = conv2d(x[n], ws[n], pad=1)         # [n, oc, h, w]

Sharding: data-parallel over batch. 8 cores x 4 samples each; routing_ws and
the expert weight bank are replicated. Everything is computed on-device.

Per-core pipeline (per sample n):
  load:    DMA x[n] fp32 -> SBUF; ACT casts to bf16 into a W-padded
           [128, 56, 58] tile, with accum_out producing the spatial sum
           (pooled) in the same pass.  Casts run two samples ahead.
  routing: PE logits[1,8] = pooled.T @ routing_ws; ACT sigmoid(/3136);
           PE ones-matmul broadcasts the 8 gates to 128 partitions; ACT copy.
           Issued one sample ahead of the conv stream.
  mixing:  wmix = sum_e g[e]*W_e per 128-channel input group. Two ACT gated
           muls seed two accumulators; DVE alternates tensor_scalar (4x) and
           tensor_tensor add (2x); final add combines.  Issued one sample
           ahead so it overlaps the previous sample's convs.
  conv:    9 taps x 2 input groups = 18 matmuls accumulate into one PSUM bank
           per (output group, 8-row tile); H edges via clipped row ranges,
           W edges via the 58-wide zero-padded layout. ACT copies PSUM->SBUF,
           DMA out.  Sample 0 runs icg0 taps for 6 banks before icg1 taps so
           the PE starts as soon as the first mixing chain lands.
"""

import numpy as np

N, C, H, W = 32, 256, 56, 56
E, OC, IC, FH, FW = 8, 256, 256, 3, 3
NCORES = 8
NS = N // NCORES          # samples per core
P = 128                   # partitions
ICG = IC // P             # input-channel groups (2)
OCG = OC // P             # output-channel groups (2)
WP = W + 2                # padded width (58)
RROWS = 8                 # output rows per PSUM tile
NRT = H // RROWS          # row tiles (7)
NTAP = FH * FW
# dy=1 taps first so the start=True matmul covers the full 8-row PSUM tile
TAP_ORDER = [(1, 0), (1, 1), (1, 2), (0, 0), (0, 1), (0, 2), (2, 0), (2, 1), (2, 2)]

_cache = {}


def _build():
    from concourse import bacc, mybir
    import concourse.tile as tile

    f32 = mybir.dt.float32
    bf16 = mybir.dt.bfloat16
    ADD = mybir.AluOpType.add
    Copy = mybir.ActivationFunctionType.Copy
    Sigmoid = mybir.ActivationFunctionType.Sigmoid

    nc = bacc.Bacc("TRN2", target_bir_lowering=False, debug=False, num_devices=NCORES)
    x_d = nc.dram_tensor("x", [NS, C, H, W], f32, kind="ExternalInput").ap()
    wt_d = nc.dram_tensor("wt", [E, ICG, P, NTAP * OC], bf16, kind="ExternalInput").ap()
    rws_d = nc.dram_tensor("rws", [ICG, P, E], f32, kind="ExternalInput").ap()
    y_d = nc.dram_tensor("y", [NS, OC, H, W], f32, kind="ExternalOutput").ap()

    with tile.TileContext(nc) as tc:
        with (
            tc.tile_pool(name="consts", bufs=1) as consts,
            tc.tile_pool(name="wbank", bufs=1) as wbank_pool,
            tc.tile_pool(name="xf", bufs=3) as xf_pool,
            tc.tile_pool(name="xb", bufs=6) as xb_pool,
            tc.tile_pool(name="small", bufs=4) as small,
            tc.tile_pool(name="wmix", bufs=4) as wmix_pool,
            tc.tile_pool(name="scratch", bufs=2) as scratch,
            tc.tile_pool(name="outp", bufs=4) as out_pool,
            tc.tile_pool(name="cpsum", bufs=6, space="PSUM") as cpsum,
            tc.tile_pool(name="rpsum", bufs=1, space="PSUM") as rpsum,
        ):
            xf_t = [None] * NS
            xb_t = [[None] * ICG for _ in range(NS)]
            pooled = [[None] * ICG for _ in range(NS)]
            g_t = [None] * NS
            wmix = [[None] * ICG for _ in range(NS)]

            def load_x(n):
                # x loads ride the ACT HWDGE ring so they run in parallel
                # with the weight-bank DMAs on the sync ring
                xf_t[n] = []
                for icg in range(ICG):
                    xf = xf_pool.tile([P, H * W], f32, name="xf")
                    nc.scalar.dma_start(xf[:], x_d[n, icg * P:(icg + 1) * P])
                    xf_t[n].append(xf)

            def cast_x(n):
                for icg in range(ICG):
                    xbt = xb_pool.tile([P, H, WP], bf16, name="xbt")
                    nc.gpsimd.memset(xbt[:, :, 0:1], 0.0)
                    nc.gpsimd.memset(xbt[:, :, W + 1:W + 2], 0.0)
                    pt = small.tile([P, 1], f32, name="pt")
                    nc.scalar.activation(
                        xbt[:, :, 1:W + 1],
                        xf_t[n][icg].rearrange("p (h w) -> p h w", h=H),
                        Copy, accum_out=pt[:],
                    )
                    xb_t[n][icg] = xbt
                    pooled[n][icg] = pt

            def routing(n):
                logits = rpsum.tile([1, E], f32, name="logits")
                for icg in range(ICG):
                    nc.tensor.matmul(
                        logits[:], pooled[n][icg][:], rws_sb[icg][:],
                        start=(icg == 0), stop=(icg == ICG - 1),
                    )
                lg = small.tile([1, E], f32, name="lg")
                nc.scalar.activation(lg[:], logits[:], Sigmoid, scale=1.0 / (H * W))
                gps = rpsum.tile([P, E], f32, name="gps")
                nc.tensor.matmul(gps[:], ones_sb[:], lg[:], start=True, stop=True)
                g = small.tile([P, E], f32, name="g")
                nc.scalar.copy(g[:], gps[:])
                g_t[n] = g

            def mixing(n, dve_only=False):
                # dve_only: latency-critical first sample — ACT seeding would
                # tie the chain to the ACT stream, which the scheduler may
                # block on unrelated DMAs. Otherwise ACT seeds 2 accumulators
                # to offload DVE throughput.
                g = g_t[n]
                for icg in range(ICG):
                    wm = wmix_pool.tile([P, NTAP * OC], bf16, name="wm")
                    accb = scratch.tile([P, NTAP * OC], bf16, name="accb")
                    if dve_only:
                        nc.vector.tensor_scalar_mul(wm[:], wbank[0][icg][:], g[:, 0:1])
                        nc.vector.tensor_scalar_mul(accb[:], wbank[1][icg][:], g[:, 1:2])
                    else:
                        nc.scalar.mul(wm[:], wbank[0][icg][:], g[:, 0:1])
                        nc.scalar.mul(accb[:], wbank[1][icg][:], g[:, 1:2])
                    for e in range(2, E):
                        p = scratch.tile([P, NTAP * OC], bf16, name="p", bufs=2)
                        nc.vector.tensor_scalar_mul(p[:], wbank[e][icg][:], g[:, e:e + 1])
                        tgt = wm if e % 2 == 0 else accb
                        nc.vector.tensor_add(tgt[:], tgt[:], p[:])
                    nc.vector.tensor_add(wm[:], wm[:], accb[:])
                    wmix[n][icg] = wm

            def conv_mms(n, ocg, rt, ps, icg, is_first, is_last):
                r0 = rt * RROWS
                cnt = 0
                for dy, dx in TAP_ORDER:
                    lo = max(r0, 1 - dy)
                    hi = min(r0 + RROWS - 1, H - dy)
                    tap = dy * FW + dx
                    nc.tensor.matmul(
                        ps[:, lo - r0:hi - r0 + 1, :],
                        wmix[n][icg][:, tap * OC + ocg * P:tap * OC + ocg * P + P],
                        xb_t[n][icg][:, lo + dy - 1:hi + dy, dx:dx + W],
                        start=(is_first and cnt == 0),
                        stop=(is_last and cnt == NTAP - 1),
                    )
                    cnt += 1

            def bank_out(n, ocg, rt, ps):
                r0 = rt * RROWS
                ot = out_pool.tile([P, RROWS, W], f32, name="ot")
                nc.scalar.copy(ot[:], ps[:])
                nc.sync.dma_start(
                    y_d[n, ocg * P:(ocg + 1) * P, r0:r0 + RROWS], ot[:]
                )

            BANKS = [(ocg, rt) for ocg in range(OCG) for rt in range(NRT)]

            def conv_banks(n, banks):
                for ocg, rt in banks:
                    ps = cpsum.tile([P, RROWS, W], f32, name="ps")
                    conv_mms(n, ocg, rt, ps, 0, True, False)
                    conv_mms(n, ocg, rt, ps, 1, False, True)
                    bank_out(n, ocg, rt, ps)

            def conv_banks_icg_split(n, banks):
                # run icg0 taps for all these banks before any icg1 taps, so
                # the PE starts as soon as the first mixing chain lands
                tiles = []
                for ocg, rt in banks:
                    ps = cpsum.tile([P, RROWS, W], f32, name="ps")
                    conv_mms(n, ocg, rt, ps, 0, True, False)
                    tiles.append(ps)
                for (ocg, rt), ps in zip(banks, tiles):
                    conv_mms(n, ocg, rt, ps, 1, False, True)
                    bank_out(n, ocg, rt, ps)

            # --- prologue ---
            # x0 halves split across both HWDGE rings so they land together;
            # the weight bank follows on the sync ring
            xf0a = xf_pool.tile([P, H * W], f32, name="xf")
            nc.scalar.dma_start(xf0a[:], x_d[0, 0:P])
            xf0b = xf_pool.tile([P, H * W], f32, name="xf")
            nc.sync.dma_start(xf0b[:], x_d[0, P:2 * P])
            xf_t[0] = [xf0a, xf0b]
            load_x(1)
            rws_sb = []
            for icg in range(ICG):
                rws_t = consts.tile([P, E], f32, name=f"rws{icg}")
                nc.sync.dma_start(rws_t[:], rws_d[icg])
                rws_sb.append(rws_t)
            ones_sb = consts.tile([1, P], f32, name="ones")
            nc.vector.memset(ones_sb[:], 1.0)
            # preload the sigmoid table set off the critical path
            warm = small.tile([1, 1], f32, name="warm")
            nc.scalar.activation(warm[:], ones_sb[0:1, 0:1], Sigmoid)
            wbank = [[None] * ICG for _ in range(E)]
            for icg in range(ICG):
                for e in range(E):
                    wb = wbank_pool.tile([P, NTAP * OC], bf16, name=f"wb{e}_{icg}")
                    nc.sync.dma_start(wb[:], wt_d[e, icg])
                    wbank[e][icg] = wb
            # routing(0)+mixing(0) must precede cast_x(1) in the ACT stream,
            # else the first sample's gates wait on sample 1's cast
            cast_x(0)
            routing(0)
            mixing(0, dve_only=True)
            cast_x(1)

            # --- main loop ---
            # First banks of convs(n) are issued before next-sample prep so
            # ACT's PSUM copies keep pace with the PE; sample 0 additionally
            # splits icg passes so convs start after one mixing chain.
            for n in range(NS):
                if n == 0:
                    conv_banks_icg_split(n, BANKS[:6])
                    head_rest = BANKS[6:]
                else:
                    conv_banks(n, BANKS[:5])
                    head_rest = BANKS[5:]
                if n + 1 < NS:
                    routing(n + 1)
                    mixing(n + 1)
                if n + 2 < NS:
                    load_x(n + 2)
                    cast_x(n + 2)
                conv_banks(n, head_rest)
    nc.finalize()
    return nc


def _get_nc():
    if "nc" not in _cache:
        _cache["nc"] = _build()
    return _cache["nc"]


def _in_maps(x, routing_ws, weight):
    import ml_dtypes

    x = np.ascontiguousarray(np.asarray(x, dtype=np.float32))
    routing_ws = np.asarray(routing_ws, dtype=np.float32)
    weight = np.asarray(weight, dtype=np.float32)
    # [E, oc, ic, fh, fw] -> [E, ic, fh, fw, oc] -> [E, ICG, 128, 9*256], bf16
    wt = np.ascontiguousarray(weight.transpose(0, 2, 3, 4, 1)).reshape(
        E, ICG, P, NTAP * OC
    ).astype(ml_dtypes.bfloat16)
    rws = np.ascontiguousarray(routing_ws.reshape(ICG, P, E))
    return [
        {"x": np.ascontiguousarray(x[i * NS:(i + 1) * NS]), "wt": wt, "rws": rws}
        for i in range(NCORES)
    ]


def _run(in_maps, **kwargs):
    from concourse import bass_utils

    return bass_utils.run_bass_kernel_spmd(
        _get_nc(), in_maps, core_ids=list(range(NCORES)), **kwargs
    )


def kernel(x, routing_ws, weight):
    res = _run(_in_maps(x, routing_ws, weight))
    return np.concatenate([r["y"] for r in res.results], axis=0)


# revision 17
# speedup vs baseline: 1.0512x; 1.0109x over previous
"""CondConv2d (soft mixture-of-experts 3x3 conv) — Trainium2 Bass kernel.

Reference computation:
    pooled  = x.mean((2,3))                      # [n, c]
    routing = sigmoid(pooled @ routing_ws)       # [n, E]
    ws      = einsum('ne,eoihw->noihw', routing, weight)
    y[n]    = conv2d(x[n], ws[n], pad=1)         # [n, oc, h, w]

Sharding: data-parallel over batch. 8 cores x 4 samples each; routing_ws and
the expert weight bank are replicated. Everything is computed on-device.

Per-core pipeline (per sample n):
  load:    DMA x[n] fp32 -> SBUF; ACT casts to bf16 into a W-padded
           [128, 56, 58] tile, with accum_out producing the spatial sum
           (pooled) in the same pass.  Casts run two samples ahead.
  routing: PE logits[1,8] = pooled.T @ routing_ws; ACT sigmoid(/3136);
           PE ones-matmul broadcasts the 8 gates to 128 partitions; ACT copy.
           Issued one sample ahead of the conv stream.
  mixing:  wmix = sum_e g[e]*W_e per 128-channel input group. Two ACT gated
           muls seed two accumulators; DVE alternates tensor_scalar (4x) and
           tensor_tensor add (2x); final add combines.  Issued one sample
           ahead so it overlaps the previous sample's convs.
  conv:    9 taps x 2 input groups = 18 matmuls accumulate into one PSUM bank
           per (output group, 8-row tile); H edges via clipped row ranges,
           W edges via the 58-wide zero-padded layout. ACT copies PSUM->SBUF,
           DMA out.  Sample 0 runs icg0 taps for 6 banks before icg1 taps so
           the PE starts as soon as the first mixing chain lands.
"""

import numpy as np

N, C, H, W = 32, 256, 56, 56
E, OC, IC, FH, FW = 8, 256, 256, 3, 3
NCORES = 8
NS = N // NCORES          # samples per core
P = 128                   # partitions
ICG = IC // P             # input-channel groups (2)
OCG = OC // P             # output-channel groups (2)
WP = W + 2                # padded width (58)
RROWS = 8                 # output rows per PSUM tile
NRT = H // RROWS          # row tiles (7)
NTAP = FH * FW
# dy=1 taps first so the start=True matmul covers the full 8-row PSUM tile
TAP_ORDER = [(1, 0), (1, 1), (1, 2), (0, 0), (0, 1), (0, 2), (2, 0), (2, 1), (2, 2)]

_cache = {}


def _build():
    from concourse import bacc, mybir
    import concourse.tile as tile

    f32 = mybir.dt.float32
    bf16 = mybir.dt.bfloat16
    ADD = mybir.AluOpType.add
    Copy = mybir.ActivationFunctionType.Copy
    Sigmoid = mybir.ActivationFunctionType.Sigmoid

    nc = bacc.Bacc("TRN2", target_bir_lowering=False, debug=False, num_devices=NCORES)
    x_d = nc.dram_tensor("x", [NS, C, H, W], f32, kind="ExternalInput").ap()
    wt_d = nc.dram_tensor("wt", [E, ICG, P, NTAP * OC], bf16, kind="ExternalInput").ap()
    rws_d = nc.dram_tensor("rws", [ICG, P, E], f32, kind="ExternalInput").ap()
    y_d = nc.dram_tensor("y", [NS, OC, H, W], f32, kind="ExternalOutput").ap()

    with tile.TileContext(nc) as tc:
        with (
            tc.tile_pool(name="consts", bufs=1) as consts,
            tc.tile_pool(name="wbank", bufs=1) as wbank_pool,
            tc.tile_pool(name="xf", bufs=3) as xf_pool,
            tc.tile_pool(name="xb", bufs=6) as xb_pool,
            tc.tile_pool(name="small", bufs=4) as small,
            tc.tile_pool(name="wmix", bufs=4) as wmix_pool,
            tc.tile_pool(name="scratch", bufs=2) as scratch,
            tc.tile_pool(name="outp", bufs=4) as out_pool,
            tc.tile_pool(name="cpsum", bufs=7, space="PSUM") as cpsum,
            tc.tile_pool(name="rpsum", bufs=1, space="PSUM") as rpsum,
        ):
            xf_t = [None] * NS
            xb_t = [[None] * ICG for _ in range(NS)]
            pooled = [[None] * ICG for _ in range(NS)]
            g_t = [None] * NS
            wmix = [[None] * ICG for _ in range(NS)]

            def load_x(n):
                # x loads ride the ACT HWDGE ring so they run in parallel
                # with the weight-bank DMAs on the sync ring
                xf_t[n] = []
                for icg in range(ICG):
                    xf = xf_pool.tile([P, H * W], f32, name="xf")
                    nc.scalar.dma_start(xf[:], x_d[n, icg * P:(icg + 1) * P])
                    xf_t[n].append(xf)

            def cast_x(n):
                for icg in range(ICG):
                    xbt = xb_pool.tile([P, H, WP], bf16, name="xbt")
                    nc.gpsimd.memset(xbt[:, :, 0:1], 0.0)
                    nc.gpsimd.memset(xbt[:, :, W + 1:W + 2], 0.0)
                    pt = small.tile([P, 1], f32, name="pt")
                    nc.scalar.activation(
                        xbt[:, :, 1:W + 1],
                        xf_t[n][icg].rearrange("p (h w) -> p h w", h=H),
                        Copy, accum_out=pt[:],
                    )
                    xb_t[n][icg] = xbt
                    pooled[n][icg] = pt

            def routing(n):
                logits = rpsum.tile([1, E], f32, name="logits", tag="rp")
                for icg in range(ICG):
                    nc.tensor.matmul(
                        logits[:], pooled[n][icg][:], rws_sb[icg][:],
                        start=(icg == 0), stop=(icg == ICG - 1),
                    )
                lg = small.tile([1, E], f32, name="lg")
                nc.scalar.activation(lg[:], logits[:], Sigmoid, scale=1.0 / (H * W))
                gps = rpsum.tile([P, E], f32, name="gps", tag="rp")
                nc.tensor.matmul(gps[:], ones_sb[:], lg[:], start=True, stop=True)
                g = small.tile([P, E], f32, name="g")
                nc.scalar.copy(g[:], gps[:])
                g_t[n] = g

            def mixing(n, dve_only=False):
                # dve_only: latency-critical first sample — ACT seeding would
                # tie the chain to the ACT stream, which the scheduler may
                # block on unrelated DMAs. Otherwise ACT seeds 2 accumulators
                # to offload DVE throughput.
                g = g_t[n]
                for icg in range(ICG):
                    wm = wmix_pool.tile([P, NTAP * OC], bf16, name="wm")
                    accb = scratch.tile([P, NTAP * OC], bf16, name="accb")
                    if dve_only and icg == 0:
                        nc.vector.tensor_scalar_mul(wm[:], wbank[0][icg][:], g[:, 0:1])
                        nc.vector.tensor_scalar_mul(accb[:], wbank[1][icg][:], g[:, 1:2])
                    else:
                        nc.scalar.mul(wm[:], wbank[0][icg][:], g[:, 0:1])
                        nc.scalar.mul(accb[:], wbank[1][icg][:], g[:, 1:2])
                    for e in range(2, E):
                        p = scratch.tile([P, NTAP * OC], bf16, name="p", bufs=2)
                        nc.vector.tensor_scalar_mul(p[:], wbank[e][icg][:], g[:, e:e + 1])
                        tgt = wm if e % 2 == 0 else accb
                        nc.vector.tensor_add(tgt[:], tgt[:], p[:])
                    nc.vector.tensor_add(wm[:], wm[:], accb[:])
                    wmix[n][icg] = wm

            def conv_mms(n, ocg, rt, ps, icg, is_first, is_last):
                r0 = rt * RROWS
                cnt = 0
                for dy, dx in TAP_ORDER:
                    lo = max(r0, 1 - dy)
                    hi = min(r0 + RROWS - 1, H - dy)
                    tap = dy * FW + dx
                    nc.tensor.matmul(
                        ps[:, lo - r0:hi - r0 + 1, :],
                        wmix[n][icg][:, tap * OC + ocg * P:tap * OC + ocg * P + P],
                        xb_t[n][icg][:, lo + dy - 1:hi + dy, dx:dx + W],
                        start=(is_first and cnt == 0),
                        stop=(is_last and cnt == NTAP - 1),
                    )
                    cnt += 1

            def bank_out(n, ocg, rt, ps):
                r0 = rt * RROWS
                ot = out_pool.tile([P, RROWS, W], f32, name="ot")
                nc.scalar.copy(ot[:], ps[:])
                nc.sync.dma_start(
                    y_d[n, ocg * P:(ocg + 1) * P, r0:r0 + RROWS], ot[:]
                )

            BANKS = [(ocg, rt) for ocg in range(OCG) for rt in range(NRT)]

            def conv_banks(n, banks):
                for ocg, rt in banks:
                    ps = cpsum.tile([P, RROWS, W], f32, name="ps")
                    conv_mms(n, ocg, rt, ps, 0, True, False)
                    conv_mms(n, ocg, rt, ps, 1, False, True)
                    bank_out(n, ocg, rt, ps)

            def conv_banks_icg_split(n, banks):
                # run icg0 taps for all these banks before any icg1 taps, so
                # the PE starts as soon as the first mixing chain lands
                tiles = []
                for ocg, rt in banks:
                    ps = cpsum.tile([P, RROWS, W], f32, name="ps")
                    conv_mms(n, ocg, rt, ps, 0, True, False)
                    tiles.append(ps)
                for (ocg, rt), ps in zip(banks, tiles):
                    conv_mms(n, ocg, rt, ps, 1, False, True)
                    bank_out(n, ocg, rt, ps)

            # --- prologue ---
            # x0 halves split across both HWDGE rings so they land together;
            # the weight bank follows on the sync ring
            xf0a = xf_pool.tile([P, H * W], f32, name="xf")
            nc.scalar.dma_start(xf0a[:], x_d[0, 0:P])
            xf0b = xf_pool.tile([P, H * W], f32, name="xf")
            nc.sync.dma_start(xf0b[:], x_d[0, P:2 * P])
            xf_t[0] = [xf0a, xf0b]
            load_x(1)
            rws_sb = []
            for icg in range(ICG):
                rws_t = consts.tile([P, E], f32, name=f"rws{icg}")
                nc.sync.dma_start(rws_t[:], rws_d[icg])
                rws_sb.append(rws_t)
            ones_sb = consts.tile([1, P], f32, name="ones")
            nc.vector.memset(ones_sb[:], 1.0)
            # preload the sigmoid table set off the critical path
            warm = small.tile([1, 1], f32, name="warm")
            nc.scalar.activation(warm[:], ones_sb[0:1, 0:1], Sigmoid)
            wbank = [[None] * ICG for _ in range(E)]
            for icg in range(ICG):
                for e in range(E):
                    wb = wbank_pool.tile([P, NTAP * OC], bf16, name=f"wb{e}_{icg}")
                    nc.sync.dma_start(wb[:], wt_d[e, icg])
                    wbank[e][icg] = wb
            # routing(0)+mixing(0) must precede cast_x(1) in the ACT stream,
            # else the first sample's gates wait on sample 1's cast
            cast_x(0)
            routing(0)
            mixing(0, dve_only=True)
            cast_x(1)

            # --- main loop ---
            # First banks of convs(n) are issued before next-sample prep so
            # ACT's PSUM copies keep pace with the PE; sample 0 additionally
            # splits icg passes so convs start after one mixing chain.
            for n in range(NS):
                if n == 0:
                    conv_banks_icg_split(n, BANKS[:6])
                    head_rest = BANKS[6:]
                else:
                    conv_banks(n, BANKS[:5])
                    head_rest = BANKS[5:]
                if n + 1 < NS:
                    routing(n + 1)
                    mixing(n + 1)
                if n + 2 < NS:
                    load_x(n + 2)
                    cast_x(n + 2)
                conv_banks(n, head_rest)
    nc.finalize()
    return nc


def _get_nc():
    if "nc" not in _cache:
        _cache["nc"] = _build()
    return _cache["nc"]


def _in_maps(x, routing_ws, weight):
    import ml_dtypes

    x = np.ascontiguousarray(np.asarray(x, dtype=np.float32))
    routing_ws = np.asarray(routing_ws, dtype=np.float32)
    weight = np.asarray(weight, dtype=np.float32)
    # [E, oc, ic, fh, fw] -> [E, ic, fh, fw, oc] -> [E, ICG, 128, 9*256], bf16
    wt = np.ascontiguousarray(weight.transpose(0, 2, 3, 4, 1)).reshape(
        E, ICG, P, NTAP * OC
    ).astype(ml_dtypes.bfloat16)
    rws = np.ascontiguousarray(routing_ws.reshape(ICG, P, E))
    return [
        {"x": np.ascontiguousarray(x[i * NS:(i + 1) * NS]), "wt": wt, "rws": rws}
        for i in range(NCORES)
    ]


def _run(in_maps, **kwargs):
    from concourse import bass_utils

    return bass_utils.run_bass_kernel_spmd(
        _get_nc(), in_maps, core_ids=list(range(NCORES)), **kwargs
    )


def kernel(x, routing_ws, weight):
    res = _run(_in_maps(x, routing_ws, weight))
    return np.concatenate([r["y"] for r in res.results], axis=0)
